# revision 1
# baseline (speedup 1.0000x reference)
"""CoordEncoder Trainium2 kernel.

Data-parallel over B across 8 NeuronCores (one batch element per core).
Per core, for its L=1024 atoms:
  q[i,j] = |x_i - x_j|^2 via one K=5 augmented matmul on PE
  d = exp(0.5*ln(relu(q)+1e-12)) on ACT (relu/ln/exp/square share one
  activation table set, natural_log_exp_and_others)
  RBF planes f_r = exp(-gamma*(d - c_r)^2), r=0..15:
    - seed planes r in {0,6,12} on ACT as Square+Exp
    - the rest chained on DVE with one tensor_tensor mult per plane:
      f~_r = f~_{r-1} * t_s, t_s = exp(2*gamma*dc*d - gamma*dc*(c_{s+1}+c_s)).
      Using the segment-constant t_s instead of the exact per-step factor
      scales plane r by the constant exp(gamma*dc^2*m*(m-1)), m = r-s,
      which the host divides out of the corresponding Wg row.
  Every plane is reduced over the neighbor axis with a PE ones-column
  matmul (column sums == row sums by symmetry of d), accumulated into a
  persistent [16, 1024] PSUM tile that is directly the lhsT of the final
  geo matmul — no transpose needed.
  out = onehot(Z)^T-gather of (atom_emb @ W1 + b) + sums^T @ Wg', on PE
  accumulated in one PSUM tile per row block.

Host side only folds weights (input-independent) and transposes layouts.
"""

import numpy as np

B, L, E, R, NA = 8, 1024, 256, 16, 118
P = 128          # partition tile
NT = L // P      # 8 i-tiles per core
SEEDS = (0, 6, 12)   # planes computed directly on ACT; chains fill the rest
GPLANES = (5, 11, 15)   # chained planes computed on GpSimd instead of DVE
BUFS = {"qc": 3, "dd": 4, "tt": 4, "sq": 3, "fb": 12, "hb": 3}
REPS = 1   # repeat the whole compute body (timing experiments only)
TSB = -5.0           # shared bias inside ts = exp(2*gamma*dc*d + TSB)
EPS = 1e-12

_CACHE = {}


def _seg_of(r):
    s = max(x for x in SEEDS if x <= r)
    return s, r - s


def _build_nc(gamma, centers, split=True):
    import concourse.bass as bass
    import concourse.tile as tile
    from concourse import mybir
    from contextlib import ExitStack

    f32 = mybir.dt.float32
    AF = mybir.ActivationFunctionType
    ALU = mybir.AluOpType

    dc = float(centers[1] - centers[0])

    nc = bass.Bass("TRN2", target_bir_lowering=False, debug=False)

    coordsT = nc.dram_tensor("coordsT", [3, L], f32, kind="ExternalInput")
    zrow = nc.dram_tensor("zrow", [1, L], f32, kind="ExternalInput")
    t1 = nc.dram_tensor("t1", [NA, E], f32, kind="ExternalInput")
    wg = nc.dram_tensor("wg", [R, E], f32, kind="ExternalInput")
    out = nc.dram_tensor("out", [L, E], f32, kind="ExternalOutput")

    with tile.TileContext(nc) as tc, ExitStack() as ctx:
        consts = ctx.enter_context(tc.tile_pool(name="consts", bufs=1))

        # ---- persistent SBUF tensors ----
        A = consts.tile([3, L], f32)            # coords^T
        t1s = consts.tile([NA, E], f32)
        wgs = consts.tile([R, E], f32)
        lhs_aug = consts.tile([5, L], f32)      # [-2x,-2y,-2z, 1, |x|^2]
        rhs_aug = consts.tile([5, L], f32)      # [x, y, z, |x|^2, 1]
        onehotT = consts.tile([NA, L], f32)     # onehot(Z)^T

        # per-partition bias constants for activation ops
        eps_b = consts.tile([P, 1], f32, tag="eps_b")
        nc.vector.memset(eps_b[:], EPS)
        cbias = {}
        for s in SEEDS:
            if s == 0:
                continue
            cb = consts.tile([P, 1], f32, tag=f"cb{s}")
            nc.vector.memset(cb[:], -float(centers[s]))
            cbias[s] = cb
        tsb = consts.tile([P, 1], f32, tag="tsb")
        nc.vector.memset(tsb[:], TSB)
        # ones-at-column-r weights for the PE column reductions (f32r for
        # full-rate PE; memset can't write f32r, so build f32 then cast-copy)
        onescol0 = consts.tile([P, R, R], f32, tag="onescol0")
        nc.vector.memset(onescol0[:], 0.0)
        for r in range(R):
            nc.vector.memset(onescol0[:, r, r:r + 1], 1.0)
        onescol = consts.tile([P, R, R], mybir.dt.float32r)
        nc.vector.tensor_copy(onescol[:], onescol0[:])

        nc.sync.dma_start(A[:], coordsT[:, :])
        nc.sync.dma_start(t1s[:], t1[:, :])
        nc.sync.dma_start(wgs[:], wg[:, :])

        # ---- setup: norms, augmented matrices, one-hot ----
        with tc.tile_pool(name="setup", bufs=1) as sp, \
             tc.tile_pool(name="setup_ps", bufs=1, space="PSUM") as spp, \
             tc.tile_pool(name="setup_dram", bufs=1, space="DRAM") as sdp:
            zs = sp.tile([1, L], f32)
            nc.sync.dma_start(zs[:], zrow[:, :])

            asq = sp.tile([3, L], f32)
            nc.vector.tensor_tensor(asq[:], A[:], A[:], ALU.mult)
            ones3 = sp.tile([3, 1], f32)
            nc.vector.memset(ones3[:], 1.0)
            nps = spp.tile([1, L], f32)
            for h in range(2):
                nc.tensor.matmul(nps[:, h * 512:(h + 1) * 512], ones3[:],
                                 asq[:, h * 512:(h + 1) * 512], start=True, stop=True)
            nsq = sp.tile([1, L], f32)
            for h in range(2):
                nc.scalar.copy(nsq[:, h * 512:(h + 1) * 512], nps[:, h * 512:(h + 1) * 512])

            # engine APs can't start at partitions 1-31, so assemble the
            # 5-row augmented matrices in DRAM scratch and load them whole
            neg2a = sp.tile([3, L], f32)
            nc.scalar.mul(neg2a[:], A[:], -2.0)
            onesrow = sp.tile([1, L], f32)
            nc.vector.memset(onesrow[:], 1.0)
            lhs_d = sdp.tile([5, L], f32)
            rhs_d = sdp.tile([5, L], f32, tag="rhs_d")
            nc.sync.dma_start(lhs_d[0:3, :], neg2a[:])
            nc.sync.dma_start(lhs_d[3:4, :], onesrow[:])
            nc.sync.dma_start(lhs_d[4:5, :], nsq[:])
            nc.sync.dma_start(rhs_d[0:3, :], A[:])
            nc.sync.dma_start(rhs_d[3:4, :], nsq[:])
            nc.sync.dma_start(rhs_d[4:5, :], onesrow[:])
            nc.sync.dma_start(lhs_aug[:], lhs_d[:])
            nc.sync.dma_start(rhs_aug[:], rhs_d[:])

            # one-hot: broadcast Z to NA partitions via K=1 matmul, compare to iota
            ones1 = sp.tile([1, NA], f32)
            nc.vector.memset(ones1[:], 1.0)
            zbp = spp.tile([NA, L], f32)
            for h in range(2):
                nc.tensor.matmul(zbp[:, h * 512:(h + 1) * 512], ones1[:],
                                 zs[:, h * 512:(h + 1) * 512], start=True, stop=True)
            iota_i = sp.tile([P, 1], mybir.dt.int32)
            nc.gpsimd.iota(iota_i[:], pattern=[[0, 1]], base=0, channel_multiplier=1)
            iota_f = sp.tile([P, 1], f32)
            nc.vector.tensor_copy(iota_f[:], iota_i[:])
            nc.vector.tensor_scalar(onehotT[:], zbp[:], iota_f[:NA, :], None, ALU.is_equal)

        # ---- main pools ----
        qpp = ctx.enter_context(tc.tile_pool(name="q_ps", bufs=2, space="PSUM"))
        cspp = ctx.enter_context(tc.tile_pool(name="cs_ps", bufs=1, space="PSUM"))
        hpp = ctx.enter_context(tc.tile_pool(name="h_ps", bufs=2, space="PSUM"))
        qcp = ctx.enter_context(tc.tile_pool(name="qc", bufs=BUFS["qc"]))
        ddp = ctx.enter_context(tc.tile_pool(name="dd", bufs=BUFS["dd"]))
        ttp = ctx.enter_context(tc.tile_pool(name="tt", bufs=BUFS["tt"]))
        sqp = ctx.enter_context(tc.tile_pool(name="sq", bufs=BUFS["sq"]))
        fbp = ctx.enter_context(tc.tile_pool(name="fb", bufs=BUFS["fb"]))
        csbp = ctx.enter_context(tc.tile_pool(name="csb", bufs=1))
        hbp = ctx.enter_context(tc.tile_pool(name="hb", bufs=BUFS["hb"]))

        for rep in range(REPS):
          cs = cspp.tile([R, L], f32)   # running sums^T for all atoms

          for it in range(NT):
            i0 = it * P
            # pairwise squared distances for this row block: [128, 1024]
            qps = qpp.tile([P, L], f32)
            for h in range(2):
                nc.tensor.matmul(qps[:, h * 512:(h + 1) * 512],
                                 lhs_aug[:, i0:i0 + P],
                                 rhs_aug[:, h * 512:(h + 1) * 512],
                                 start=True, stop=True)
            qc = qcp.tile([P, L], f32)
            for h in range(2):
                nc.scalar.activation(qc[:, h * 512:(h + 1) * 512],
                                     qps[:, h * 512:(h + 1) * 512], AF.Relu)
            lq = qcp.tile([P, L], f32, tag="lq")
            nc.scalar.activation(lq[:], qc[:], AF.Ln, bias=eps_b[:])
            dd = ddp.tile([P, L], f32)
            nc.scalar.activation(dd[:], lq[:], AF.Exp, scale=0.5)
            ts = ttp.tile([P, L], f32)
            nc.scalar.activation(ts[:], dd[:], AF.Exp,
                                 scale=2.0 * gamma * dc, bias=tsb[:])

            planes = {}
            for s in SEEDS:
                if s == 0:
                    fs = fbp.tile([P, L], mybir.dt.float32r)
                    nc.scalar.activation(fs[:], qc[:], AF.Exp, scale=-gamma)
                else:
                    sq = sqp.tile([P, L], f32)
                    nc.scalar.activation(sq[:], dd[:], AF.Square, bias=cbias[s][:])
                    fs = fbp.tile([P, L], mybir.dt.float32r)
                    nc.scalar.activation(fs[:], sq[:], AF.Exp, scale=-gamma)
                planes[s] = fs
                f = fs
                end = min(s + 6, R)
                for r in range(s + 1, end):
                    fn = fbp.tile([P, L], mybir.dt.float32r)
                    eng = nc.gpsimd if r in GPLANES else nc.vector
                    eng.tensor_tensor(fn[:], f[:], ts[:], ALU.mult)
                    planes[r] = fn
                    f = fn
            # reduce every plane over its 128 rows into cs (col sums == row
            # sums by symmetry), accumulating across row blocks on PE
            for r in range(R):
                f = planes[r]
                for h in range(2):
                    # float32r streams at full rate for N>=256 (fp32 is 1/4);
                    # safe here: plain sums of positives, no cancellation
                    nc.tensor.matmul(cs[:, h * 512:(h + 1) * 512],
                                     onescol[:, r, :].bitcast(mybir.dt.float32r),
                                     f[:, h * 512:(h + 1) * 512].bitcast(mybir.dt.float32r),
                                     start=(it == 0 and r == 0),
                                     stop=(it == NT - 1 and r == R - 1),
                                     skip_group_check=True)

          # ---- tail: project and emit ----
          csb = csbp.tile([R, L], f32)
          for h in range(2):
            nc.vector.tensor_copy(csb[:, h * 512:(h + 1) * 512], cs[:, h * 512:(h + 1) * 512])
          for it in range(NT):
            i0 = it * P
            hps = hpp.tile([P, E], f32)
            nc.tensor.matmul(hps[:], onehotT[:, i0:i0 + P], t1s[:],
                             start=True, stop=False, skip_group_check=True)
            nc.tensor.matmul(hps[:], csb[:, i0:i0 + P], wgs[:],
                             start=False, stop=True, skip_group_check=True)
            hb = hbp.tile([P, E], f32)
            nc.vector.tensor_copy(hb[:], hps[:])
            nc.sync.dma_start(out[i0:i0 + P, :], hb[:])

    if split:
        _split_excess_waits(nc)
    return nc


def _split_excess_waits(nc, maxw=1):
    """This walrus build rejects instructions carrying more than one sem wait
    (setupSyncWait: 'Too many sync wait commands'). Move excess waits onto
    injected same-engine NOPs that execute immediately before."""
    from concourse import mybir
    n = 0
    for fn in nc.m.functions:
        for bb in fn.blocks:
            new = []
            for ins in bb.instructions:
                si = ins.sync_info
                if si is not None and si.on_wait and len(si.on_wait) > maxw:
                    waits = list(si.on_wait)
                    excess, keep = waits[:-maxw], waits[-maxw:]
                    for ci in range(0, len(excess), maxw):
                        nop = mybir.InstNoOp(name=f"waitsplit_{ins.name}_{ci}",
                                             ins=[], outs=[])
                        nop.engine = ins.engine
                        nop.bass_nofuse = True
                        nop.sync_info = mybir.SyncInfo(on_wait=excess[ci:ci + maxw],
                                                       on_update=[])
                        new.append(nop)
                        n += 1
                    si.on_wait = keep
                new.append(ins)
            bb.instructions[:] = new
    return n


def _prep_inputs(coords, Z, atom_emb, rbf_centers, gamma, rbf_proj_w,
                 rbf_proj_b, out_proj_w, out_proj_b):
    f64 = np.float64
    g = float(np.asarray(gamma))
    centers = np.asarray(rbf_centers, dtype=f64)
    dc = float(centers[1] - centers[0])
    w1 = np.asarray(out_proj_w)[:E].astype(f64)
    w2 = np.asarray(out_proj_w)[E:].astype(f64)
    bias = (np.asarray(rbf_proj_b).astype(f64) @ w2) + np.asarray(out_proj_b).astype(f64)
    t1 = (np.asarray(atom_emb).astype(f64) @ w1 + bias).astype(np.float32)
    wgm = (np.asarray(rbf_proj_w).astype(f64) @ w2) / L
    # divide out the chain drift from using the shared ts (see _build_nc)
    for r in range(R):
        s, m = _seg_of(r)
        if m:
            wgm[r] /= np.exp(m * TSB + g * dc * (2 * centers[s] * m + dc * m * m))
    wgm = wgm.astype(np.float32)
    in_maps = []
    for b in range(B):
        in_maps.append({
            "coordsT": np.ascontiguousarray(np.asarray(coords)[b].T).astype(np.float32),
            "zrow": np.asarray(Z)[b].astype(np.float32).reshape(1, L),
            "t1": t1, "wg": wgm,
        })
    return in_maps


def _get_nc(gamma, centers):
    key = (float(gamma),) + tuple(float(c) for c in centers)
    if key not in _CACHE:
        _CACHE[key] = _build_nc(float(gamma), [float(c) for c in centers])
    return _CACHE[key]


def _run(in_maps, gamma, centers, trace=False):
    from concourse.bass_utils import run_bass_kernel_spmd
    nc = _get_nc(gamma, centers)
    return run_bass_kernel_spmd(nc, in_maps, core_ids=list(range(B)), trace=trace)


def kernel(coords, Z, atom_emb, rbf_centers, gamma, rbf_proj_w, rbf_proj_b,
           out_proj_w, out_proj_b):
    centers = np.asarray(rbf_centers, dtype=np.float64)
    steps = np.diff(centers)
    assert np.allclose(steps, steps[0], rtol=1e-5), "uniform RBF grid expected"
    in_maps = _prep_inputs(coords, Z, atom_emb, rbf_centers, gamma, rbf_proj_w,
                           rbf_proj_b, out_proj_w, out_proj_b)
    res = _run(in_maps, float(np.asarray(gamma)), centers)
    return np.stack([res.results[b]["out"] for b in range(B)], axis=0)

